# revision 1
# baseline (speedup 1.0000x reference)
"""GNN message-passing kernel for 8 Trainium2 NeuronCores.

Reference computation:
    msg = x[edge_index[1]]                       # [E, 64] gather
    out = segment_sum(msg, edge_index[0], N)     # [N, 64] scatter-add

Strategy (edge-parallel, node-sharded output):
  - Destination nodes are sharded across 8 cores (12500 nodes/core).
  - Each core processes exactly the edges whose dst lies in its shard.
  - Gather x[src] with the custom SWDGE `dma_gather` (256B rows from HBM).
    int16 index limit (32768) -> src space split into 4 windows; edges are
    grouped by (round, window) on the host.
  - Scatter-add into the core's HBM output shard with `dma_scatter_add`.
    The CCE read-modify-write races on duplicate indices within a call, so
    edges are partitioned into "rounds": round r holds at most one edge per
    destination node (rank-r edge of each node).  Rounds are serialized via
    the DMA completion semaphore; different rounds never race.
  - All index prep on host is pure index manipulation (no float math).

SPMD: one Bass program runs on all 8 cores, so all segment capacities are
made uniform across cores (max over cores, padded).  Pad gather slots point
at row 0 of the window (harmless real data) and pad scatter slots target a
dummy output row that is dropped on the host.
"""

import functools

import numpy as np

import concourse.bacc as bacc
import concourse.bass as bass
import concourse.mybir as mybir
from concourse.bass_utils import run_bass_kernel_spmd

N_NODES = 100000
D = 64
N_CORES = 8
SHARD = N_NODES // N_CORES  # 12500
WIN = 32768  # int16 index window
N_WIN = (N_NODES + WIN - 1) // WIN  # 4
CHUNK_SLOTS = 24576  # msg slots per chunk buffer (x2 buffers = 12.6MB SBUF)
OUT_ROWS = ((SHARD + 1 + 127) // 128) * 128  # 12544; row SHARD is the dummy
MAX_CALL = 1024  # >=~2048 idxs in one SWDGE call overflows the desc ring

_f32 = mybir.dt.float32
_i16 = mybir.dt.int16


def _host_prep(edge_index):
    """Partition/order edges; build per-core int16 index arrays.

    Returns (cfg, per_core_arrays) where cfg is hashable compile config.
    """
    dst = np.asarray(edge_index[0]).astype(np.int64)
    src = np.asarray(edge_index[1]).astype(np.int64)

    cores = []
    for c in range(N_CORES):
        m = (dst >= c * SHARD) & (dst < (c + 1) * SHARD)
        dstc = (dst[m] - c * SHARD).astype(np.int32)
        srcc = src[m].astype(np.int32)
        # sort by dst; rank within each dst node = scatter round
        o = np.argsort(dstc, kind="stable")
        dstc, srcc = dstc[o], srcc[o]
        counts = np.bincount(dstc, minlength=SHARD)
        starts = np.concatenate(([0], np.cumsum(counts)[:-1]))
        rank = (np.arange(dstc.size) - starts[dstc]).astype(np.int32)
        win = (srcc // WIN).astype(np.int32)
        cores.append((dstc, srcc, rank, win))

    n_rounds = max(int(pc[2].max()) + 1 for pc in cores)

    # capacity per (round, win) = max over cores, padded to 128
    caps = np.zeros((n_rounds, N_WIN), dtype=np.int64)
    for dstc, srcc, rank, win in cores:
        key = rank * N_WIN + win
        cnt = np.bincount(key, minlength=n_rounds * N_WIN)
        caps = np.maximum(caps, cnt.reshape(n_rounds, N_WIN))
    caps = ((caps + 127) // 128) * 128

    seg_off = np.zeros((n_rounds, N_WIN), dtype=np.int64)
    np.cumsum(caps.ravel()[:-1], out=seg_off.ravel()[1:])
    s_tot = int(caps.sum())

    # fill per-core slot arrays
    per_core = []
    for dstc, srcc, rank, win in cores:
        src_loc = np.zeros(s_tot, dtype=np.int16)
        dst_loc = np.full(s_tot, SHARD, dtype=np.int16)
        key = rank * N_WIN + win
        o2 = np.argsort(key, kind="stable")
        key_s = key[o2]
        seg_counts = np.bincount(key_s, minlength=n_rounds * N_WIN)
        seg_starts = np.concatenate(([0], np.cumsum(seg_counts)[:-1]))
        slot = seg_off.ravel()[key_s] + (np.arange(key_s.size) - seg_starts[key_s])
        src_loc[slot] = (srcc[o2] & (WIN - 1)).astype(np.int16)
        dst_loc[slot] = dstc[o2].astype(np.int16)
        # wrap for the q7 index layout: idx j lives at [j % 16, j // 16]
        per_core.append(
            {
                "srcloc": np.ascontiguousarray(src_loc.reshape(-1, 16).T),
                "dstloc": np.ascontiguousarray(dst_loc.reshape(-1, 16).T),
            }
        )

    # chunk rounds so each chunk's slots fit one msg buffer
    round_len = caps.sum(axis=1)
    chunks = []  # list of (first_round, last_round_exclusive)
    r0 = 0
    while r0 < n_rounds:
        r1 = r0
        tot = 0
        while r1 < n_rounds and tot + round_len[r1] <= CHUNK_SLOTS:
            tot += round_len[r1]
            r1 += 1
        assert r1 > r0, f"round {r0} too large for chunk buffer"
        chunks.append((r0, r1))
        r0 = r1

    cfg = (
        tuple(map(tuple, caps.tolist())),
        tuple(chunks),
        s_tot,
    )
    return cfg, per_core


@functools.lru_cache(maxsize=4)
def _build(cfg):
    caps, chunks, s_tot = cfg
    caps = np.asarray(caps, dtype=np.int64)
    n_rounds = caps.shape[0]
    seg_off = np.zeros((n_rounds, N_WIN), dtype=np.int64)
    np.cumsum(caps.ravel()[:-1], out=seg_off.ravel()[1:])
    round_off = np.concatenate(([0], np.cumsum(caps.sum(axis=1))))

    nc = bacc.Bacc(None, num_swdge_queues=2)
    x_t = nc.dram_tensor("x", [N_NODES, D], _f32, kind="ExternalInput")
    src_t = nc.dram_tensor("srcloc", [16, s_tot // 16], _i16, kind="ExternalInput")
    dst_t = nc.dram_tensor("dstloc", [16, s_tot // 16], _i16, kind="ExternalInput")
    out_t = nc.dram_tensor("out", [OUT_ROWS, D], _f32, kind="ExternalOutput")

    cs128 = CHUNK_SLOTS // 128

    with (
        nc.sbuf_tensor([128, s_tot // 16], _i16) as src_sb,
        nc.sbuf_tensor([128, s_tot // 16], _i16) as dst_sb,
        nc.sbuf_tensor([128, 2 * cs128, D], _f32) as msg_sb,
        nc.sbuf_tensor([128, OUT_ROWS * D // 128], _f32) as zero_sb,
        nc.semaphore("gsem0") as gsem0,
        nc.semaphore("gsem1") as gsem1,
        nc.semaphore("isem") as isem,
        nc.semaphore("ssem") as ssem,
        nc.semaphore("psem") as psem,
        nc.semaphore("msem") as msem,
        nc.Block() as block,
    ):

        @block.gpsimd
        def _(g):
            cnt_reg = nc.alloc_register(mybir.EngineType.Pool, "cnt")

            def creg(n):
                g.reg_mov(cnt_reg, n)
                return cnt_reg

            # --- preamble: load index arrays, zero the output shard
            g.memset(src_sb[:], 0).then_inc(msem, 1)
            g.memset(dst_sb[:], 0).then_inc(msem, 1)
            g.memset(zero_sb[:], 0.0).then_inc(msem, 1)
            g.wait_ge(msem, 3)
            for p0 in range(0, 64, 16):
                g.dma_start(src_sb[p0 : p0 + 16, :], src_t[:]).then_inc(isem, 16)
                g.dma_start(dst_sb[p0 : p0 + 16, :], dst_t[:]).then_inc(isem, 16)
            out_zview = out_t.rearrange("(p a) d -> p (a d)", p=128)
            g.dma_start(out_zview, zero_sb[:]).then_inc(isem, 16)
            g.wait_ge(isem, 16 * 9)

            gsems = (gsem0, gsem1)
            cum_g = [0, 0]  # cumulative gather count per parity sem
            chunk_gwait = {}  # ci -> value to wait on gsems[ci % 2]
            n_prepped = 0
            calls_in_round = [0] * n_rounds
            calls_through = {0: 0}  # round r -> scatter calls before round r

            def issue_gathers(ci):
                r0, r1 = chunks[ci]
                buf = (ci % 2) * cs128
                base = round_off[r0]
                for r in range(r0, r1):
                    for w in range(N_WIN):
                        cap = int(caps[r, w])
                        if cap == 0:
                            continue
                        hi_row = min((w + 1) * WIN, N_NODES)
                        for sub in range(0, cap, MAX_CALL):
                            n = min(MAX_CALL, cap - sub)
                            off = int(seg_off[r, w]) + sub
                            lo = buf + (off - base) // 128
                            g.dma_gather(
                                msg_sb[:, lo : lo + n // 128, :],
                                x_t[w * WIN : hi_row, :],
                                src_sb[:, off // 16 : (off + n) // 16],
                                n,
                                creg(n),
                                D,
                                queue_num=0,
                            ).then_inc(gsems[ci % 2], 16)
                            cum_g[ci % 2] += 1
                chunk_gwait[ci] = 16 * cum_g[ci % 2]

            def plan_scatters(ci):
                r0, r1 = chunks[ci]
                for r in range(r0, r1):
                    rlen = int(round_off[r + 1] - round_off[r])
                    calls_in_round[r] = (rlen + MAX_CALL - 1) // MAX_CALL
                    calls_through[r + 1] = calls_through[r] + calls_in_round[r]

            def fire_rounds(ci):
                r0, r1 = chunks[ci]
                buf = (ci % 2) * cs128
                base = round_off[r0]
                # all of this chunk's gathers have landed (completions on one
                # SWDGE queue are out-of-order; wait for the whole chunk)
                g.wait_ge(gsems[ci % 2], chunk_gwait[ci])
                for r in range(r0, r1):
                    # all previous rounds complete (WAW safety; within a
                    # round dsts are unique so its calls may fly together)
                    g.wait_ge(ssem, 16 * calls_through[r])
                    rlen = int(round_off[r + 1] - round_off[r])
                    for sub in range(0, rlen, MAX_CALL):
                        n = min(MAX_CALL, rlen - sub)
                        off = int(round_off[r]) + sub
                        lo = buf + (off - base) // 128
                        g.dma_scatter_add(
                            out_t[:],
                            msg_sb[:, lo : lo + n // 128, :],
                            dst_sb[:, off // 16 : (off + n) // 16],
                            n,
                            creg(n),
                            D,
                            queue_num=1,
                        ).then_inc(ssem, 16)

            for ci in range(len(chunks)):
                plan_scatters(ci)
            issue_gathers(0)
            for ci in range(1, len(chunks)):
                if ci >= 2:
                    # msg buffer reuse: chunk ci-2's scatters must be done
                    pr = chunks[ci - 2][1]
                    g.wait_ge(ssem, 16 * calls_through[pr])
                issue_gathers(ci)
                fire_rounds(ci - 1)
            fire_rounds(len(chunks) - 1)
            g.wait_ge(ssem, 16 * calls_through[n_rounds])

    nc.finalize()
    return nc


def kernel(x, edge_index):
    x = np.ascontiguousarray(np.asarray(x), dtype=np.float32)
    cfg, per_core = _host_prep(edge_index)
    nc = _build(cfg)
    in_maps = [
        {"x": x, "srcloc": pc["srcloc"], "dstloc": pc["dstloc"]} for pc in per_core
    ]
    res = run_bass_kernel_spmd(nc, in_maps, list(range(N_CORES)))
    out = np.concatenate([res.results[c]["out"][:SHARD] for c in range(N_CORES)])
    return out.astype(np.float32)



# revision 2
# speedup vs baseline: 1.0462x; 1.0462x over previous
"""GNN message-passing kernel v2 for 8 Trainium2 NeuronCores.

Reference:  msg = x[edge_index[1]]; out = segment_sum(msg, edge_index[0], N).

v1 profiling showed the bottleneck is the GpSimd q7 SWDGE ucode at ~10ns per
descriptor, serialized on one engine for BOTH the gather (125k/core) and the
scatter-add (125k/core).  v2 keeps the SWDGE gather but deletes the scatter:
the segment-sum is done on-chip by the TensorEngine.

Layout (per core, dst rows [c*12500, (c+1)*12500)):
  - window w = 128 consecutive output rows; 98 windows (12544 padded rows).
  - Tokens (edges) sorted by dst; within a window grouped by src int16-window
    s (src//32768, 4 of them) so each (w, s) run is one gather call range.
  - Slot capacity per (w, s) = 128*B[w,s] with B = max over cores (SPMD: one
    program for all cores).  Pad slots gather x[s*32768] and carry segid -1.
  - Token j sits at msg[p= j%128, col=j//128, :] (dma_gather layout).
  - A block = one msg column = 128 tokens, all belonging to window w.
    DVE builds A[tok, seg] = (segid[tok] == iota[seg]) as f32 one-hot.
    PE: psum[w%8] (+)= A_block.T @ msg_block, start/stop per window.
  - DVE evacuates psum -> staging; ACT writes staging rows sequentially to
    DRAM (static DMA).  No CCE scatter, no RMW, no zeroing pass.

Pipeline: GpSimd gathers chunk ci+1 while DVE/PE/ACT process chunk ci
(chunk = 8 windows).  A-strip single-buffered, msg+staging double-buffered.
"""

import contextlib
import functools

import numpy as np

import concourse.bacc as bacc
import concourse.bass as bass
import concourse.mybir as mybir
from concourse.bass_utils import run_bass_kernel_spmd

N_NODES = 100000
D = 64
N_CORES = 8
SHARD = N_NODES // N_CORES      # 12500
N_WINDOWS = (SHARD + 127) // 128  # 98
OUT_ROWS = N_WINDOWS * 128      # 12544
SRC_WIN = 32768
N_SRCW = (N_NODES + SRC_WIN - 1) // SRC_WIN  # 4
MAX_CALL = 1024
CHUNK_W = 8                     # windows per chunk

_f32 = mybir.dt.float32
_i16 = mybir.dt.int16


def _host_prep(edge_index):
    dst = np.asarray(edge_index[0]).astype(np.int64)
    src = np.asarray(edge_index[1]).astype(np.int64)

    cores = []
    cnts = np.zeros((N_CORES, N_WINDOWS * N_SRCW), dtype=np.int64)
    for c in range(N_CORES):
        m = (dst >= c * SHARD) & (dst < (c + 1) * SHARD)
        dstc = (dst[m] - c * SHARD).astype(np.int32)
        srcc = src[m].astype(np.int32)
        key = ((dstc >> 7) * N_SRCW + (srcc // SRC_WIN)).astype(np.int64)
        o = np.argsort(key, kind="stable")
        dstc, srcc, key = dstc[o], srcc[o], key[o]
        cnts[c] = np.bincount(key, minlength=N_WINDOWS * N_SRCW)
        cores.append((dstc, srcc, key))

    B = -(-cnts.max(axis=0) // 128).reshape(N_WINDOWS, N_SRCW)
    B[:, 0] = np.maximum(B[:, 0], 1)  # ensure >=1 block per window
    caps = B * 128
    seg_off = np.zeros(N_WINDOWS * N_SRCW, dtype=np.int64)
    np.cumsum(caps.ravel()[:-1], out=seg_off[1:])
    s_tot = int(caps.sum())

    per_core = []
    for dstc, srcc, key in cores:
        src_loc = np.zeros(s_tot, dtype=np.int16)
        seg = np.full(s_tot, -1.0, dtype=np.float32)
        seg_counts = np.bincount(key, minlength=N_WINDOWS * N_SRCW)
        seg_starts = np.concatenate(([0], np.cumsum(seg_counts)[:-1]))
        slot = seg_off[key] + (np.arange(key.size) - seg_starts[key])
        src_loc[slot] = (srcc & (SRC_WIN - 1)).astype(np.int16)
        seg[slot] = (dstc & 127).astype(np.float32)
        per_core.append(
            {
                "srcloc": np.ascontiguousarray(
                    src_loc.reshape(-1, 16).T
                ),  # [16, s_tot/16], idx j at [j%16, j//16]
                "segid": np.ascontiguousarray(
                    seg.reshape(-1, 128).T
                ),  # [128, s_tot/128], slot j at [j%128, j//128]
            }
        )

    cfg = (tuple(map(tuple, B.tolist())), s_tot)
    return cfg, per_core


@functools.lru_cache(maxsize=4)
def _build(cfg):
    Bt, s_tot = cfg
    B = np.asarray(Bt, dtype=np.int64)          # [W, S]
    caps = B * 128
    seg_off = np.zeros(N_WINDOWS * N_SRCW, dtype=np.int64)
    np.cumsum(caps.ravel()[:-1], out=seg_off[1:])
    seg_off = seg_off.reshape(N_WINDOWS, N_SRCW)

    # chunk structure
    chunks = []  # (w0, w1)
    for w0 in range(0, N_WINDOWS, CHUNK_W):
        chunks.append((w0, min(w0 + CHUNK_W, N_WINDOWS)))
    nch = len(chunks)

    def chunk_base(ci):
        return int(seg_off[chunks[ci][0], 0])

    def chunk_end(ci):
        w1 = chunks[ci][1]
        return int(seg_off[w1, 0]) if w1 < N_WINDOWS else s_tot

    maxch = max((chunk_end(ci) - chunk_base(ci)) // 128 for ci in range(nch))

    # gather calls per chunk: list of (abs_off, n)
    calls = [[] for _ in range(nch)]
    for ci, (w0, w1) in enumerate(chunks):
        for w in range(w0, w1):
            for s in range(N_SRCW):
                cap = int(caps[w, s])
                if cap == 0:
                    continue
                for sub in range(0, cap, MAX_CALL):
                    calls[ci].append(
                        (int(seg_off[w, s]) + sub, min(MAX_CALL, cap - sub), s)
                    )
    calls_through = np.cumsum([len(c) for c in calls]).tolist()
    win_through = [chunks[ci][1] for ci in range(nch)]

    nc = bacc.Bacc(None, num_swdge_queues=2)
    x_t = nc.dram_tensor("x", [N_NODES, D], _f32, kind="ExternalInput")
    src_t = nc.dram_tensor("srcloc", [16, s_tot // 16], _i16, kind="ExternalInput")
    seg_t = nc.dram_tensor("segid", [128, s_tot // 128], _f32, kind="ExternalInput")
    iota_t = nc.dram_tensor("iota", [128, 128], _f32, kind="ExternalInput")
    out_t = nc.dram_tensor("out", [OUT_ROWS, D], _f32, kind="ExternalOutput")

    with (
        contextlib.ExitStack() as _ps,
        nc.sbuf_tensor([128, s_tot // 16], _i16) as src_sb,
        nc.sbuf_tensor([128, s_tot // 128], _f32) as seg_sb,
        nc.sbuf_tensor([128, 128], _f32) as iota_sb,
        nc.sbuf_tensor([128, 2 * maxch, D], _f32) as msg_sb,
        nc.sbuf_tensor([128, maxch, 128], _f32) as a_sb,
        nc.sbuf_tensor([128, 2 * CHUNK_W, D], _f32) as stag_sb,
        nc.semaphore("s_pre") as s_pre,
        nc.semaphore("s_g0") as s_g0,
        nc.semaphore("s_g1") as s_g1,
        nc.semaphore("s_a") as s_a,
        nc.semaphore("s_w") as s_w,
        nc.semaphore("s_e") as s_e,
        nc.semaphore("s_out") as s_out,
        nc.Block() as block,
    ):
        psums = [
            _ps.enter_context(nc.psum_tensor(f"psw{_i}", [128, D], _f32))
            for _i in range(CHUNK_W)
        ]

        @block.sync
        def _(e):
            for p0 in range(0, 128, 16):
                e.dma_start(src_sb[p0 : p0 + 16, :], src_t[:]).then_inc(s_pre, 16)
            e.dma_start(seg_sb[:], seg_t[:]).then_inc(s_pre, 16)
            e.dma_start(iota_sb[:], iota_t[:]).then_inc(s_pre, 16)

        gsems = (s_g0, s_g1)
        # cumulative gather-call count per parity after each chunk
        gwait = []
        _cum = [0, 0]
        for _ci in range(nch):
            _cum[_ci % 2] += len(calls[_ci])
            gwait.append(16 * _cum[_ci % 2])

        @block.gpsimd
        def _(g):
            cnt_reg = nc.alloc_register(mybir.EngineType.Pool, "cnt")
            g.wait_ge(s_pre, 16 * 10)
            for ci in range(nch):
                if ci >= 2:
                    g.wait_ge(s_w, win_through[ci - 2])  # PE done, msg buf free
                base = chunk_base(ci)
                buf = (ci % 2) * maxch
                for off, n, s in calls[ci]:
                    hi = min((s + 1) * SRC_WIN, N_NODES)
                    lo = buf + (off - base) // 128
                    g.reg_mov(cnt_reg, n)
                    g.dma_gather(
                        msg_sb[:, lo : lo + n // 128, :],
                        x_t[s * SRC_WIN : hi, :],
                        src_sb[:, off // 16 : (off + n) // 16],
                        n,
                        cnt_reg,
                        D,
                        queue_num=0,
                    ).then_inc(gsems[ci % 2], 16)

        @block.vector
        def _(v):
            v.wait_ge(s_pre, 16 * 10)
            for ci in range(nch):
                # build A strip for chunk ci (buffer reused from ci-1)
                if ci >= 1:
                    v.wait_ge(s_w, win_through[ci - 1])  # PE consumed strip ci-1
                base = chunk_base(ci)
                ncols = (chunk_end(ci) - base) // 128
                for lc in range(ncols):
                    gcol = base // 128 + lc
                    ins = v.tensor_tensor(
                        out=a_sb[:, lc, :],
                        in0=seg_sb[:, gcol : gcol + 1].to_broadcast([128, 128]),
                        in1=iota_sb[:],
                        op=mybir.AluOpType.is_equal,
                    )
                    if lc == ncols - 1:
                        ins.then_inc(s_a, 1)
                # evacuate previous chunk's psum windows
                if ci >= 1:
                    pj = ci - 1
                    if pj >= 2:
                        v.wait_ge(s_out, 16 * (pj - 1))  # stag slot free
                    w0, w1 = chunks[pj]
                    for w in range(w0, w1):
                        v.wait_ge(s_w, w + 1)
                        v.tensor_copy(
                            stag_sb[:, (pj % 2) * CHUNK_W + (w - w0), :],
                            psums[w % CHUNK_W][:],
                        ).then_inc(s_e, 1)
            # final chunk evac
            pj = nch - 1
            if pj >= 2:
                v.wait_ge(s_out, 16 * (pj - 1))
            w0, w1 = chunks[pj]
            for w in range(w0, w1):
                v.wait_ge(s_w, w + 1)
                v.tensor_copy(
                    stag_sb[:, (pj % 2) * CHUNK_W + (w - w0), :],
                    psums[w % CHUNK_W][:],
                ).then_inc(s_e, 1)

        @block.tensor
        def _(t):
            for ci in range(nch):
                t.wait_ge(s_a, ci + 1)
                t.wait_ge(gsems[ci % 2], gwait[ci])
                base = chunk_base(ci)
                buf = (ci % 2) * maxch
                w0, w1 = chunks[ci]
                for w in range(w0, w1):
                    if w >= CHUNK_W:
                        t.wait_ge(s_e, w - CHUNK_W + 1)
                    nb = int(B[w].sum())
                    c0 = (int(seg_off[w, 0]) - base) // 128
                    for j in range(nb):
                        ins = t.matmul(
                            out=psums[w % CHUNK_W][:],
                            lhsT=a_sb[:, c0 + j, :],
                            rhs=msg_sb[:, buf + c0 + j, :],
                            start=(j == 0),
                            stop=(j == nb - 1),
                        )
                        if j == nb - 1:
                            ins.then_inc(s_w, 1)

        @block.scalar
        def _(a):
            for ci in range(nch):
                if ci >= 2:
                    a.wait_ge(s_out, 16 * (ci - 1))
                a.wait_ge(s_e, win_through[ci])
                w0, w1 = chunks[ci]
                nw = w1 - w0
                view = out_t[w0 * 128 : w1 * 128, :].rearrange(
                    "(a p) d -> p a d", p=128
                )
                a.dma_start(
                    view, stag_sb[:, (ci % 2) * CHUNK_W : (ci % 2) * CHUNK_W + nw, :]
                ).then_inc(s_out, 16)
            a.wait_ge(s_out, 16 * nch)

    nc.finalize()
    return nc


def _iota_arr():
    return np.broadcast_to(
        np.arange(128, dtype=np.float32), (128, 128)
    ).copy()


def kernel(x, edge_index):
    x = np.ascontiguousarray(np.asarray(x), dtype=np.float32)
    cfg, per_core = _host_prep(edge_index)
    nc = _build(cfg)
    iota = _iota_arr()
    in_maps = [
        {"x": x, "srcloc": pc["srcloc"], "segid": pc["segid"], "iota": iota}
        for pc in per_core
    ]
    res = run_bass_kernel_spmd(nc, in_maps, list(range(N_CORES)))
    out = np.concatenate([res.results[c]["out"][:SHARD] for c in range(N_CORES)])
    return out.astype(np.float32)


# revision 3
# speedup vs baseline: 1.0467x; 1.0005x over previous
"""GNN message-passing kernel v2 for 8 Trainium2 NeuronCores.

Reference:  msg = x[edge_index[1]]; out = segment_sum(msg, edge_index[0], N).

v1 profiling showed the bottleneck is the GpSimd q7 SWDGE ucode at ~10ns per
descriptor, serialized on one engine for BOTH the gather (125k/core) and the
scatter-add (125k/core).  v2 keeps the SWDGE gather but deletes the scatter:
the segment-sum is done on-chip by the TensorEngine.

Layout (per core, dst rows [c*12500, (c+1)*12500)):
  - window w = 128 consecutive output rows; 98 windows (12544 padded rows).
  - Tokens (edges) sorted by dst; within a window grouped by src int16-window
    s (src//32768, 4 of them) so each (w, s) run is one gather call range.
  - Slot capacity per (w, s) = 128*B[w,s] with B = max over cores (SPMD: one
    program for all cores).  Pad slots gather x[s*32768] and carry segid -1.
  - Token j sits at msg[p= j%128, col=j//128, :] (dma_gather layout).
  - A block = one msg column = 128 tokens, all belonging to window w.
    DVE builds A[tok, seg] = (segid[tok] == iota[seg]) as f32 one-hot.
    PE: psum[w%8] (+)= A_block.T @ msg_block, start/stop per window.
  - DVE evacuates psum -> staging; ACT writes staging rows sequentially to
    DRAM (static DMA).  No CCE scatter, no RMW, no zeroing pass.

Pipeline: GpSimd gathers chunk ci+1 while DVE/PE/ACT process chunk ci
(chunk = 8 windows).  A-strip single-buffered, msg+staging double-buffered.
"""

import contextlib
import functools

import numpy as np

import concourse.bacc as bacc
import concourse.bass as bass
import concourse.mybir as mybir
from concourse.bass_utils import run_bass_kernel_spmd

N_NODES = 100000
D = 64
N_CORES = 8
SHARD = N_NODES // N_CORES      # 12500
N_WINDOWS = (SHARD + 127) // 128  # 98
OUT_ROWS = N_WINDOWS * 128      # 12544
SRC_WIN = 32768
N_SRCW = (N_NODES + SRC_WIN - 1) // SRC_WIN  # 4
MAX_CALL = 1024
CHUNK_W = 4                     # windows per chunk

_f32 = mybir.dt.float32
_i16 = mybir.dt.int16


def _host_prep(edge_index):
    dst = np.asarray(edge_index[0]).astype(np.int64)
    src = np.asarray(edge_index[1]).astype(np.int64)

    cores = []
    cnts = np.zeros((N_CORES, N_WINDOWS * N_SRCW), dtype=np.int64)
    for c in range(N_CORES):
        m = (dst >= c * SHARD) & (dst < (c + 1) * SHARD)
        dstc = (dst[m] - c * SHARD).astype(np.int32)
        srcc = src[m].astype(np.int32)
        key = ((dstc >> 7) * N_SRCW + (srcc // SRC_WIN)).astype(np.int64)
        o = np.argsort(key, kind="stable")
        dstc, srcc, key = dstc[o], srcc[o], key[o]
        cnts[c] = np.bincount(key, minlength=N_WINDOWS * N_SRCW)
        cores.append((dstc, srcc, key))

    B = -(-cnts.max(axis=0) // 128).reshape(N_WINDOWS, N_SRCW)
    B[:, 0] = np.maximum(B[:, 0], 1)  # ensure >=1 block per window
    caps = B * 128
    run_off = np.zeros(N_WINDOWS * N_SRCW, dtype=np.int64)
    np.cumsum(caps.ravel()[:-1], out=run_off[1:])
    s_tot = int(caps.sum())

    per_core = []
    for dstc, srcc, key in cores:
        src_loc = np.zeros(s_tot, dtype=np.int16)
        seg = np.full(s_tot, -1.0, dtype=np.float32)
        seg_counts = np.bincount(key, minlength=N_WINDOWS * N_SRCW)
        seg_starts = np.concatenate(([0], np.cumsum(seg_counts)[:-1]))
        slot = run_off[key] + (np.arange(key.size) - seg_starts[key])
        src_loc[slot] = (srcc & (SRC_WIN - 1)).astype(np.int16)
        seg[slot] = (dstc & 127).astype(np.float32)
        per_core.append(
            {
                "srcloc": np.ascontiguousarray(
                    src_loc.reshape(-1, 16).T
                ),  # [16, s_tot/16], idx j at [j%16, j//16]
                "segid": np.ascontiguousarray(
                    seg.reshape(-1, 128).T
                ),  # [128, s_tot/128], slot j at [j%128, j//128]
            }
        )

    cfg = (tuple(map(tuple, B.tolist())), s_tot)
    return cfg, per_core


@functools.lru_cache(maxsize=4)
def _build(cfg):
    Bt, s_tot = cfg
    B = np.asarray(Bt, dtype=np.int64)          # [W, S]
    caps = B * 128
    run_off = np.zeros(N_WINDOWS * N_SRCW, dtype=np.int64)
    np.cumsum(caps.ravel()[:-1], out=run_off[1:])
    run_off = run_off.reshape(N_WINDOWS, N_SRCW)
    capw = caps.sum(axis=1)
    capw_off = run_off[:, 0]

    # chunk structure
    chunks = []  # (w0, w1)
    for w0 in range(0, N_WINDOWS, CHUNK_W):
        chunks.append((w0, min(w0 + CHUNK_W, N_WINDOWS)))
    nch = len(chunks)

    def chunk_base(ci):
        return int(capw_off[chunks[ci][0]])

    def chunk_end(ci):
        w1 = chunks[ci][1]
        return int(capw_off[w1]) if w1 < N_WINDOWS else s_tot

    maxch = max((chunk_end(ci) - chunk_base(ci)) // 128 for ci in range(nch))

    # gather calls per chunk: list of (abs_off, n)
    calls = [[] for _ in range(nch)]
    for ci, (w0, w1) in enumerate(chunks):
        for w in range(w0, w1):
            for s in range(N_SRCW):
                cap = int(caps[w, s])
                if cap == 0:
                    continue
                for sub in range(0, cap, MAX_CALL):
                    calls[ci].append(
                        (int(run_off[w, s]) + sub, min(MAX_CALL, cap - sub), s)
                    )
    calls_through = np.cumsum([len(c) for c in calls]).tolist()
    win_through = [chunks[ci][1] for ci in range(nch)]

    nc = bacc.Bacc(None, num_swdge_queues=2)
    x_t = nc.dram_tensor("x", [N_NODES, D], _f32, kind="ExternalInput")
    src_t = nc.dram_tensor("srcloc", [16, s_tot // 16], _i16, kind="ExternalInput")
    seg_t = nc.dram_tensor("segid", [128, s_tot // 128], _f32, kind="ExternalInput")
    iota_t = nc.dram_tensor("iota", [128, 128], _f32, kind="ExternalInput")
    out_t = nc.dram_tensor("out", [OUT_ROWS, D], _f32, kind="ExternalOutput")

    with (
        contextlib.ExitStack() as _ps,
        nc.sbuf_tensor([128, s_tot // 16], _i16) as src_sb,
        nc.sbuf_tensor([128, s_tot // 128], _f32) as seg_sb,
        nc.sbuf_tensor([128, 128], _f32) as iota_sb,
        nc.sbuf_tensor([128, 2 * maxch, D], _f32) as msg_sb,
        nc.sbuf_tensor([128, maxch, 128], _f32) as a_sb,
        nc.sbuf_tensor([128, 2 * CHUNK_W, D], _f32) as stag_sb,
        nc.semaphore("s_pre") as s_pre,
        nc.semaphore("s_g0") as s_g0,
        nc.semaphore("s_g1") as s_g1,
        nc.semaphore("s_a") as s_a,
        nc.semaphore("s_w") as s_w,
        nc.semaphore("s_e") as s_e,
        nc.semaphore("s_out") as s_out,
        nc.Block() as block,
    ):
        psums = [
            _ps.enter_context(nc.psum_tensor(f"psw{_i}", [128, D], _f32))
            for _i in range(CHUNK_W)
        ]

        @block.sync
        def _(e):
            for p0 in range(0, 128, 16):
                e.dma_start(src_sb[p0 : p0 + 16, :], src_t[:]).then_inc(s_pre, 16)
            e.dma_start(seg_sb[:], seg_t[:]).then_inc(s_pre, 16)
            e.dma_start(iota_sb[:], iota_t[:]).then_inc(s_pre, 16)

        gsems = (s_g0, s_g1)
        # cumulative gather-call count per parity after each chunk
        gwait = []
        _cum = [0, 0]
        for _ci in range(nch):
            _cum[_ci % 2] += len(calls[_ci])
            gwait.append(16 * _cum[_ci % 2])

        @block.gpsimd
        def _(g):
            cnt_reg = nc.alloc_register(mybir.EngineType.Pool, "cnt")
            g.wait_ge(s_pre, 16 * 10 + 1)
            for ci in range(nch):
                if ci >= 2:
                    g.wait_ge(s_w, win_through[ci - 2])  # PE done, msg buf free
                base = chunk_base(ci)
                buf = (ci % 2) * maxch
                for off, n, s in calls[ci]:
                    hi = min((s + 1) * SRC_WIN, N_NODES)
                    lo = buf + (off - base) // 128
                    g.reg_mov(cnt_reg, n)
                    g.dma_gather(
                        msg_sb[:, lo : lo + n // 128, :],
                        x_t[s * SRC_WIN : hi, :],
                        src_sb[:, off // 16 : (off + n) // 16],
                        n,
                        cnt_reg,
                        D,
                        queue_num=0,
                    ).then_inc(gsems[ci % 2], 16)

        @block.vector
        def _(v):
            v.memset(msg_sb[:], 0.0).then_inc(s_pre, 1)
            v.wait_ge(s_pre, 16 * 10 + 1)
            for ci in range(nch):
                # build A strip for chunk ci (buffer reused from ci-1)
                if ci >= 1:
                    v.wait_ge(s_w, win_through[ci - 1])  # PE consumed strip ci-1
                base = chunk_base(ci)
                ncols = (chunk_end(ci) - base) // 128
                for lc in range(ncols):
                    gcol = base // 128 + lc
                    ins = v.tensor_tensor(
                        out=a_sb[:, lc, :],
                        in0=seg_sb[:, gcol : gcol + 1].to_broadcast([128, 128]),
                        in1=iota_sb[:],
                        op=mybir.AluOpType.is_equal,
                    )
                    if lc == ncols - 1:
                        ins.then_inc(s_a, 1)
                # evacuate previous chunk's psum windows
                if ci >= 1:
                    pj = ci - 1
                    if pj >= 2:
                        v.wait_ge(s_out, 16 * (pj - 1))  # stag slot free
                    w0, w1 = chunks[pj]
                    for w in range(w0, w1):
                        v.wait_ge(s_w, w + 1)
                        v.tensor_copy(
                            stag_sb[:, (pj % 2) * CHUNK_W + (w - w0), :],
                            psums[w % CHUNK_W][:],
                        ).then_inc(s_e, 1)
            # final chunk evac
            pj = nch - 1
            if pj >= 2:
                v.wait_ge(s_out, 16 * (pj - 1))
            w0, w1 = chunks[pj]
            for w in range(w0, w1):
                v.wait_ge(s_w, w + 1)
                v.tensor_copy(
                    stag_sb[:, (pj % 2) * CHUNK_W + (w - w0), :],
                    psums[w % CHUNK_W][:],
                ).then_inc(s_e, 1)

        @block.tensor
        def _(t):
            for ci in range(nch):
                t.wait_ge(s_a, ci + 1)
                t.wait_ge(gsems[ci % 2], gwait[ci])
                base = chunk_base(ci)
                buf = (ci % 2) * maxch
                w0, w1 = chunks[ci]
                for w in range(w0, w1):
                    if w >= CHUNK_W:
                        t.wait_ge(s_e, w - CHUNK_W + 1)
                    nb = int(capw[w]) // 128
                    c0 = (int(capw_off[w]) - base) // 128
                    for j in range(nb):
                        ins = t.matmul(
                            out=psums[w % CHUNK_W][:],
                            lhsT=a_sb[:, c0 + j, :],
                            rhs=msg_sb[:, buf + c0 + j, :],
                            start=(j == 0),
                            stop=(j == nb - 1),
                        )
                        if j == nb - 1:
                            ins.then_inc(s_w, 1)

        @block.scalar
        def _(a):
            for ci in range(nch):
                if ci >= 2:
                    a.wait_ge(s_out, 16 * (ci - 1))
                a.wait_ge(s_e, win_through[ci])
                w0, w1 = chunks[ci]
                nw = w1 - w0
                view = out_t[w0 * 128 : w1 * 128, :].rearrange(
                    "(a p) d -> p a d", p=128
                )
                a.dma_start(
                    view, stag_sb[:, (ci % 2) * CHUNK_W : (ci % 2) * CHUNK_W + nw, :]
                ).then_inc(s_out, 16)
            a.wait_ge(s_out, 16 * nch)

    nc.finalize()
    return nc


def _iota_arr():
    return np.broadcast_to(
        np.arange(128, dtype=np.float32), (128, 128)
    ).copy()


def kernel(x, edge_index):
    x = np.ascontiguousarray(np.asarray(x), dtype=np.float32)
    cfg, per_core = _host_prep(edge_index)
    nc = _build(cfg)
    iota = _iota_arr()
    in_maps = [
        {"x": x, "srcloc": pc["srcloc"], "segid": pc["segid"], "iota": iota}
        for pc in per_core
    ]
    res = run_bass_kernel_spmd(nc, in_maps, list(range(N_CORES)))
    out = np.concatenate([res.results[c]["out"][:SHARD] for c in range(N_CORES)])
    return out.astype(np.float32)


# revision 4
# speedup vs baseline: 1.1551x; 1.1035x over previous
"""GNN message-passing kernel v2 for 8 Trainium2 NeuronCores.

Reference:  msg = x[edge_index[1]]; out = segment_sum(msg, edge_index[0], N).

v1 profiling showed the bottleneck is the GpSimd q7 SWDGE ucode at ~10ns per
descriptor, serialized on one engine for BOTH the gather (125k/core) and the
scatter-add (125k/core).  v2 keeps the SWDGE gather but deletes the scatter:
the segment-sum is done on-chip by the TensorEngine.

Layout (per core, dst rows [c*12500, (c+1)*12500)):
  - window w = 128 consecutive output rows; 98 windows (12544 padded rows).
  - Tokens (edges) sorted by dst; within a window grouped by src int16-window
    s (src//32768, 4 of them) so each (w, s) run is one gather call range.
  - Slot capacity per (w, s) = 128*B[w,s] with B = max over cores (SPMD: one
    program for all cores).  Pad slots gather x[s*32768] and carry segid -1.
  - Token j sits at msg[p= j%128, col=j//128, :] (dma_gather layout).
  - A block = one msg column = 128 tokens, all belonging to window w.
    DVE builds A[tok, seg] = (segid[tok] == iota[seg]) as f32 one-hot.
    PE: psum[w%8] (+)= A_block.T @ msg_block, start/stop per window.
  - DVE evacuates psum -> staging; ACT writes staging rows sequentially to
    DRAM (static DMA).  No CCE scatter, no RMW, no zeroing pass.

Pipeline: GpSimd gathers chunk ci+1 while DVE/PE/ACT process chunk ci
(chunk = 8 windows).  A-strip single-buffered, msg+staging double-buffered.
"""

import contextlib
import functools

import numpy as np

import concourse.bacc as bacc
import concourse.bass as bass
import concourse.mybir as mybir
from concourse.bass_utils import run_bass_kernel_spmd

N_NODES = 100000
D = 64
N_CORES = 8
SHARD = N_NODES // N_CORES      # 12500
N_WINDOWS = (SHARD + 127) // 128  # 98
OUT_ROWS = N_WINDOWS * 128      # 12544
SRC_WIN = 32768
N_SRCW = (N_NODES + SRC_WIN - 1) // SRC_WIN  # 4
MAX_CALL = 1024
CHUNK_W = 4                     # windows per chunk

_f32 = mybir.dt.float32
_i16 = mybir.dt.int16


def _host_prep(edge_index):
    dst = np.asarray(edge_index[0]).astype(np.int64)
    src = np.asarray(edge_index[1]).astype(np.int64)

    cores = []
    cnts = np.zeros((N_CORES, N_WINDOWS * N_SRCW), dtype=np.int64)
    for c in range(N_CORES):
        m = (dst >= c * SHARD) & (dst < (c + 1) * SHARD)
        dstc = (dst[m] - c * SHARD).astype(np.int32)
        srcc = src[m].astype(np.int32)
        key = ((dstc >> 7) * N_SRCW + (srcc // SRC_WIN)).astype(np.int64)
        o = np.argsort(key, kind="stable")
        dstc, srcc, key = dstc[o], srcc[o], key[o]
        cnts[c] = np.bincount(key, minlength=N_WINDOWS * N_SRCW)
        cores.append((dstc, srcc, key))

    B = -(-cnts.max(axis=0) // 128).reshape(N_WINDOWS, N_SRCW)
    B[:, 0] = np.maximum(B[:, 0], 1)  # ensure >=1 block per window
    caps = B * 128
    run_off = np.zeros(N_WINDOWS * N_SRCW, dtype=np.int64)
    np.cumsum(caps.ravel()[:-1], out=run_off[1:])
    s_tot = int(caps.sum())

    per_core = []
    for dstc, srcc, key in cores:
        src_loc = np.zeros(s_tot, dtype=np.int16)
        seg = np.full(s_tot, -1.0, dtype=np.float32)
        seg_counts = np.bincount(key, minlength=N_WINDOWS * N_SRCW)
        seg_starts = np.concatenate(([0], np.cumsum(seg_counts)[:-1]))
        slot = run_off[key] + (np.arange(key.size) - seg_starts[key])
        src_loc[slot] = (srcc & (SRC_WIN - 1)).astype(np.int16)
        seg[slot] = (dstc & 127).astype(np.float32)
        per_core.append(
            {
                "srcloc": np.ascontiguousarray(
                    src_loc.reshape(-1, 16).T
                ),  # [16, s_tot/16], idx j at [j%16, j//16]
                "segid": np.ascontiguousarray(
                    seg.reshape(-1, 128).T
                ),  # [128, s_tot/128], slot j at [j%128, j//128]
            }
        )

    cfg = (tuple(map(tuple, B.tolist())), s_tot)
    return cfg, per_core


@functools.lru_cache(maxsize=4)
def _build(cfg):
    Bt, s_tot = cfg
    B = np.asarray(Bt, dtype=np.int64)          # [W, S]
    caps = B * 128
    run_off = np.zeros(N_WINDOWS * N_SRCW, dtype=np.int64)
    np.cumsum(caps.ravel()[:-1], out=run_off[1:])
    run_off = run_off.reshape(N_WINDOWS, N_SRCW)
    capw = caps.sum(axis=1)
    capw_off = run_off[:, 0]

    # chunk structure
    chunks = []  # (w0, w1)
    for w0 in range(0, N_WINDOWS, CHUNK_W):
        chunks.append((w0, min(w0 + CHUNK_W, N_WINDOWS)))
    nch = len(chunks)

    def chunk_base(ci):
        return int(capw_off[chunks[ci][0]])

    def chunk_end(ci):
        w1 = chunks[ci][1]
        return int(capw_off[w1]) if w1 < N_WINDOWS else s_tot

    maxch = max((chunk_end(ci) - chunk_base(ci)) // 128 for ci in range(nch))

    # gather calls per chunk: list of (abs_off, n)
    calls = [[] for _ in range(nch)]
    for ci, (w0, w1) in enumerate(chunks):
        for w in range(w0, w1):
            for s in range(N_SRCW):
                cap = int(caps[w, s])
                if cap == 0:
                    continue
                for sub in range(0, cap, MAX_CALL):
                    calls[ci].append(
                        (int(run_off[w, s]) + sub, min(MAX_CALL, cap - sub), s)
                    )
    calls_through = np.cumsum([len(c) for c in calls]).tolist()
    win_through = [chunks[ci][1] for ci in range(nch)]

    nc = bacc.Bacc(None, num_swdge_queues=2)
    x_t = nc.dram_tensor("x", [N_NODES, D], _f32, kind="ExternalInput")
    src_t = nc.dram_tensor("srcloc", [16, s_tot // 16], _i16, kind="ExternalInput")
    seg_t = nc.dram_tensor("segid", [128, s_tot // 128], _f32, kind="ExternalInput")
    iota_t = nc.dram_tensor("iota", [128, 128], _f32, kind="ExternalInput")
    out_t = nc.dram_tensor("out", [OUT_ROWS, D], _f32, kind="ExternalOutput")

    with (
        contextlib.ExitStack() as _ps,
        nc.sbuf_tensor([128, s_tot // 16], _i16) as src_sb,
        nc.sbuf_tensor([128, s_tot // 128], _f32) as seg_sb,
        nc.sbuf_tensor([128, 128], _f32) as iota_sb,
        nc.sbuf_tensor([128, 2 * maxch, D], _f32) as msg_sb,
        nc.sbuf_tensor([128, maxch, 128], _f32) as a_sb,
        nc.sbuf_tensor([128, 2 * CHUNK_W, D], _f32) as stag_sb,
        nc.semaphore("s_pre") as s_pre,
        nc.semaphore("s_pre2") as s_pre2,
        nc.semaphore("s_g0") as s_g0,
        nc.semaphore("s_g1") as s_g1,
        nc.semaphore("s_a") as s_a,
        nc.semaphore("s_w") as s_w,
        nc.semaphore("s_e") as s_e,
        nc.semaphore("s_out") as s_out,
        nc.Block() as block,
    ):
        psums = [
            _ps.enter_context(nc.psum_tensor(f"psw{_i}", [128, D], _f32))
            for _i in range(CHUNK_W)
        ]

        @block.sync
        def _(e):
            for p0 in range(0, 128, 16):
                e.dma_start(src_sb[p0 : p0 + 16, :], src_t[:]).then_inc(s_pre, 16)
            e.dma_start(seg_sb[:], seg_t[:]).then_inc(s_pre2, 16)
            e.dma_start(iota_sb[:], iota_t[:]).then_inc(s_pre2, 16)

        gsems = (s_g0, s_g1)
        # cumulative gather-call count per parity after each chunk
        gwait = []
        _cum = [0, 0]
        for _ci in range(nch):
            _cum[_ci % 2] += len(calls[_ci])
            gwait.append(16 * _cum[_ci % 2])

        @block.gpsimd
        def _(g):
            cnt_reg = nc.alloc_register(mybir.EngineType.Pool, "cnt")
            g.wait_ge(s_pre, 16 * 8)
            for ci in range(nch):
                if ci >= 2:
                    g.wait_ge(s_w, win_through[ci - 2])  # PE done, msg buf free
                base = chunk_base(ci)
                buf = (ci % 2) * maxch
                for off, n, s in calls[ci]:
                    hi = min((s + 1) * SRC_WIN, N_NODES)
                    lo = buf + (off - base) // 128
                    g.reg_mov(cnt_reg, n)
                    g.dma_gather(
                        msg_sb[:, lo : lo + n // 128, :],
                        x_t[s * SRC_WIN : hi, :],
                        src_sb[:, off // 16 : (off + n) // 16],
                        n,
                        cnt_reg,
                        D,
                        queue_num=0,
                    ).then_inc(gsems[ci % 2], 16)

        @block.vector
        def _(v):
            v.wait_ge(s_pre2, 16 * 2)
            for ci in range(nch):
                # build A strip for chunk ci (buffer reused from ci-1)
                if ci >= 1:
                    v.wait_ge(s_w, win_through[ci - 1])  # PE consumed strip ci-1
                base = chunk_base(ci)
                ncols = (chunk_end(ci) - base) // 128
                for lc in range(ncols):
                    gcol = base // 128 + lc
                    ins = v.tensor_tensor(
                        out=a_sb[:, lc, :],
                        in0=seg_sb[:, gcol : gcol + 1].to_broadcast([128, 128]),
                        in1=iota_sb[:],
                        op=mybir.AluOpType.is_equal,
                    )
                    if lc == ncols - 1:
                        ins.then_inc(s_a, 1)
                # evacuate previous chunk's psum windows
                if ci >= 1:
                    pj = ci - 1
                    if pj >= 2:
                        v.wait_ge(s_out, 16 * (pj - 1))  # stag slot free
                    w0, w1 = chunks[pj]
                    for w in range(w0, w1):
                        v.wait_ge(s_w, w + 1)
                        v.tensor_copy(
                            stag_sb[:, (pj % 2) * CHUNK_W + (w - w0), :],
                            psums[w % CHUNK_W][:],
                        ).then_inc(s_e, 1)
            # final chunk evac
            pj = nch - 1
            if pj >= 2:
                v.wait_ge(s_out, 16 * (pj - 1))
            w0, w1 = chunks[pj]
            for w in range(w0, w1):
                v.wait_ge(s_w, w + 1)
                v.tensor_copy(
                    stag_sb[:, (pj % 2) * CHUNK_W + (w - w0), :],
                    psums[w % CHUNK_W][:],
                ).then_inc(s_e, 1)

        @block.tensor
        def _(t):
            for ci in range(nch):
                t.wait_ge(s_a, ci + 1)
                t.wait_ge(gsems[ci % 2], gwait[ci])
                base = chunk_base(ci)
                buf = (ci % 2) * maxch
                w0, w1 = chunks[ci]
                for w in range(w0, w1):
                    if w >= CHUNK_W:
                        t.wait_ge(s_e, w - CHUNK_W + 1)
                    nb = int(capw[w]) // 128
                    c0 = (int(capw_off[w]) - base) // 128
                    for j in range(nb):
                        ins = t.matmul(
                            out=psums[w % CHUNK_W][:],
                            lhsT=a_sb[:, c0 + j, :],
                            rhs=msg_sb[:, buf + c0 + j, :],
                            start=(j == 0),
                            stop=(j == nb - 1),
                        )
                        if j == nb - 1:
                            ins.then_inc(s_w, 1)

        @block.scalar
        def _(a):
            for ci in range(nch):
                if ci >= 2:
                    a.wait_ge(s_out, 16 * (ci - 1))
                a.wait_ge(s_e, win_through[ci])
                w0, w1 = chunks[ci]
                nw = w1 - w0
                view = out_t[w0 * 128 : w1 * 128, :].rearrange(
                    "(a p) d -> p a d", p=128
                )
                a.dma_start(
                    view, stag_sb[:, (ci % 2) * CHUNK_W : (ci % 2) * CHUNK_W + nw, :]
                ).then_inc(s_out, 16)
            a.wait_ge(s_out, 16 * nch)

    nc.finalize()
    return nc


def _iota_arr():
    return np.broadcast_to(
        np.arange(128, dtype=np.float32), (128, 128)
    ).copy()


def kernel(x, edge_index):
    x = np.ascontiguousarray(np.asarray(x), dtype=np.float32)
    cfg, per_core = _host_prep(edge_index)
    nc = _build(cfg)
    iota = _iota_arr()
    in_maps = [
        {"x": x, "srcloc": pc["srcloc"], "segid": pc["segid"], "iota": iota}
        for pc in per_core
    ]
    res = run_bass_kernel_spmd(nc, in_maps, list(range(N_CORES)))
    out = np.concatenate([res.results[c]["out"][:SHARD] for c in range(N_CORES)])
    return out.astype(np.float32)
